# revision 10
# baseline (speedup 1.0000x reference)
"""Trainium2 Bass kernel for the Dynamic MultiTeacher distillation loss.

Strategy v4 (data-parallel over 8 NeuronCores, 1024 rows each), device
data in bf16 (host converts; tolerance is 2e-2, device noise ~1e-4):

Per 128-row tile:
  - DMA: s, x1..x4 bf16 (halves HBM traffic).
  - PE: mimS = x1+x2+x3+x4, psd1 = x1-s, psd2 = x2-s (PSUM f32 via
    identity matmuls; exact on bf16 inputs).
  - ACT: the 7 exp passes (only engine with exp); all 7 row-sums ride
    as fused accums (SS1, SS20, S1..S4, Sm).
  - DVE: top-2: x1/x2 via pairwise-max folds + max8(250), x3/x4/mimS
    via direct max8; dots D1..D4, DmA as single-pass stt-accum; DmB as
    tensor_scalar accum over the Pool-built em*s product.
  - Pool (GPSIMD, SBUF add/sub/mult only): diff3, diff4, em*s product.

All SBUF tiles are padded to 64B-multiple pitches so every ring buffer
lands on the same alignment class (DVE 2x/4x perf modes are
alignment-sensitive).

Host (tiny O(B) work in f64): exact gathers from the f32 originals,
global min/max scalars, margins = relu(gathered - m2), threshold
softmax, KD_t = T*D_t/S_t + T^2*(lse_s - lse_t), CE, final mean.
"""

import numpy as np

N_CORES = 8
B_FULL = 8192
C_DIM = 1000
CP = 1024                          # padded tile pitch (elements)
B_LOC = B_FULL // N_CORES          # 1024 rows per core
P = 128                            # partitions
N_TILES = B_LOC // P               # 8 row-tiles per core

T_KD = 20.0
T_THR = 6.0
EPS = 1e-05

# device output column layout: [P, 55] f32 (tile padded to 64 cols)
#   cols 8t..8t+7 : top8 of stream t (t=0..3: x_{t+1}; t=4: mimS=4*mimic)
#   40 SS1  = sum exp(s)
#   41 SS20 = sum exp(s/20)
#   42..45  : S_t = sum exp(x_t/20), t=1..4
#   46 Sm   = sum exp(mimS/80)
#   47..50  : D_t = sum e_t*(x_t - s), t=1..4
#   51 DmA  = sum em*mimS      (host: D_m = DmA/4 - DmB)
#   52 DmB  = sum em*s
OUT_COLS = 55

_CACHE = {}


def _build_nc():
    import concourse.bacc as bacc
    import concourse.mybir as mybir
    from concourse import tile

    nc = bacc.Bacc(
        "TRN2",
        target_bir_lowering=False,
        debug=False,
        num_devices=N_CORES,
    )
    f32 = mybir.dt.float32
    bf16 = mybir.dt.bfloat16
    Alu = mybir.AluOpType
    Act = mybir.ActivationFunctionType

    xs = [
        nc.dram_tensor(f"x{t}", [B_LOC, C_DIM], bf16, kind="ExternalInput").ap()
        for t in range(4)
    ]
    s_dram = nc.dram_tensor("s", [B_LOC, C_DIM], bf16, kind="ExternalInput").ap()
    ident = nc.dram_tensor("ident", [P, P], bf16, kind="ExternalInput").ap()
    negid = nc.dram_tensor("negid", [P, P], bf16, kind="ExternalInput").ap()
    res = nc.dram_tensor("res", [B_LOC, OUT_COLS], f32, kind="ExternalOutput").ap()

    with tile.TileContext(nc) as tc:
        with (
            tc.tile_pool(name="const", bufs=1) as cpool,
            tc.tile_pool(name="io", bufs=3) as xpool,
            tc.tile_pool(name="exps", bufs=2) as epool,
            tc.tile_pool(name="work", bufs=2) as wpool,
            tc.tile_pool(name="sink", bufs=2) as spool,
            tc.tile_pool(name="outs", bufs=3) as opool,
            tc.tile_pool(name="psm", bufs=2, space="PSUM") as pmpool,
            tc.tile_pool(name="psd", bufs=1, space="PSUM") as psdpool,
        ):
            id_tile = cpool.tile([P, P], bf16, tag="id")
            nc.sync.dma_start(out=id_tile[:], in_=ident)
            nid_tile = cpool.tile([P, P], bf16, tag="nid")
            nc.sync.dma_start(out=nid_tile[:], in_=negid)

            for i in range(N_TILES):
                r0 = i * P
                st_ = xpool.tile([P, CP], bf16, tag="s")
                st = st_[:, 0:C_DIM]
                nc.sync.dma_start(out=st, in_=s_dram[r0 : r0 + P, :])
                xt = []
                for t in range(4):
                    x_ = xpool.tile([P, CP], bf16, tag=f"x{t}")
                    x = x_[:, 0:C_DIM]
                    nc.sync.dma_start(out=x, in_=xs[t][r0 : r0 + P, :])
                    xt.append(x)

                out_t = opool.tile([P, 64], f32)

                # ---- ACT: student lse sums (only need s) ----
                sink1 = spool.tile([P, CP], bf16, tag="sink1")
                nc.scalar.activation(
                    sink1[:, 0:C_DIM], st, Act.Exp, scale=1.0,
                    accum_out=out_t[:, 40:41],
                )
                sink2 = spool.tile([P, CP], bf16, tag="sink2")
                nc.scalar.activation(
                    sink2[:, 0:C_DIM], st, Act.Exp, scale=1.0 / T_KD,
                    accum_out=out_t[:, 41:42],
                )

                # ---- PE: psd1 = x1-s, psd2 = x2-s, mimS = sum(x) ----
                psd1 = psdpool.tile([P, C_DIM], f32, tag="psd1")
                mims = pmpool.tile([P, C_DIM], f32, tag="mims")
                for c0, c1 in ((0, 512), (512, C_DIM)):
                    nc.tensor.matmul(
                        psd1[:, c0:c1], id_tile[:], xt[0][:, c0:c1],
                        start=True, stop=False,
                    )
                    nc.tensor.matmul(
                        psd1[:, c0:c1], nid_tile[:], st[:, c0:c1],
                        start=False, stop=True,
                    )
                for c0, c1 in ((0, 512), (512, C_DIM)):
                    for t in range(4):
                        nc.tensor.matmul(
                            mims[:, c0:c1], id_tile[:], xt[t][:, c0:c1],
                            start=(t == 0), stop=(t == 3),
                        )

                # ---- ACT: teacher exps (all carry accums) ----
                e_tiles = []
                for t in range(4):
                    e_ = epool.tile([P, CP], bf16, tag=f"e{t}")
                    e = e_[:, 0:C_DIM]
                    nc.scalar.activation(
                        e, xt[t], Act.Exp, scale=1.0 / T_KD,
                        accum_out=out_t[:, 42 + t : 43 + t],
                    )
                    e_tiles.append(e)
                em_ = epool.tile([P, CP], bf16, tag="em")
                em = em_[:, 0:C_DIM]
                nc.scalar.activation(
                    em, mims[:], Act.Exp, scale=1.0 / (4.0 * T_KD),
                    accum_out=out_t[:, 46:47],
                )

                # ---- DVE: top-2 via direct max8 ----
                for t in range(4):
                    nc.vector.max(out=out_t[:, 8 * t : 8 * t + 8], in_=xt[t])
                nc.vector.max(out=out_t[:, 32:40], in_=mims[:])

                # ---- Pool: diffs for teachers 2,3,4 (SBUF tt only) ----
                diffs = {}
                for t in (1, 2, 3):
                    d_ = wpool.tile([P, CP], bf16, tag=f"df{t}")
                    d = d_[:, 0:C_DIM]
                    nc.gpsimd.tensor_tensor(
                        out=d, in0=xt[t], in1=st, op=Alu.subtract
                    )
                    diffs[t] = d

                # ---- DVE: dots ----
                junk_ = wpool.tile([P, CP], bf16, tag="junk")
                junk = junk_[:, 0:C_DIM]
                nc.vector.scalar_tensor_tensor(
                    out=junk, in0=e_tiles[0], scalar=0.0,
                    in1=psd1[:], op0=Alu.bypass, op1=Alu.mult,
                    accum_out=out_t[:, 47:48],
                )
                for t in (1, 2, 3):
                    nc.vector.scalar_tensor_tensor(
                        out=junk, in0=e_tiles[t], scalar=0.0,
                        in1=diffs[t], op0=Alu.bypass, op1=Alu.mult,
                        accum_out=out_t[:, 47 + t : 48 + t],
                    )
                nc.vector.scalar_tensor_tensor(
                    out=junk, in0=em, scalar=0.0,
                    in1=mims[:], op0=Alu.bypass, op1=Alu.mult,
                    accum_out=out_t[:, 51:52],
                )
                nc.vector.scalar_tensor_tensor(
                    out=junk, in0=em, scalar=0.0,
                    in1=st, op0=Alu.bypass, op1=Alu.mult,
                    accum_out=out_t[:, 52:53],
                )

                nc.sync.dma_start(out=res[r0 : r0 + P, :],
                                  in_=out_t[:, 0:OUT_COLS])

    nc.finalize()
    return nc


def _get_nc():
    if "nc" not in _CACHE:
        _CACHE["nc"] = _build_nc()
    return _CACHE["nc"]


def _run_device(in_maps, trace=False):
    from concourse.bass_utils import run_bass_kernel_spmd

    nc = _get_nc()
    return run_bass_kernel_spmd(
        nc, in_maps, core_ids=list(range(N_CORES)), trace=trace
    )


def _host_combine(res_cores, g, g_s):
    """res_cores: [N_CORES][B_LOC, OUT_COLS] f32; g: [B,4] gathered teacher
    logits (f64); g_s: [B] gathered student logits (f64)."""
    r = np.concatenate(res_cores, axis=0).astype(np.float64)  # [B, 55]

    g_m = g.mean(axis=1)                                     # mimic gathered
    gathered = np.concatenate([g, g_m[:, None]], axis=1)     # [B,5]

    m1 = r[:, [0, 8, 16, 24, 32]].copy()
    m2 = r[:, [1, 9, 17, 25, 33]].copy()
    m1[:, 4] *= 0.25
    m2[:, 4] *= 0.25

    SS1 = r[:, 40]
    SS20 = r[:, 41]
    S = r[:, 42:47]                                          # [B,5]
    D = np.empty((r.shape[0], 5))
    D[:, :4] = r[:, 47:51]
    D[:, 4] = r[:, 51] * 0.25 - r[:, 52]

    Cmin = g.min()
    shift = (-Cmin + EPS) if Cmin < 0 else 0.0

    margins = np.maximum(gathered - m2, 0.0)
    z = margins / T_THR
    z = z - z.max(axis=1, keepdims=True)
    ez = np.exp(z)
    thr = ez / ez.sum(axis=1, keepdims=True)

    max_preds = m1[:, :4].max() + shift

    lse_t = np.log(S)
    KD = T_KD * D / S + (T_KD * T_KD) * (np.log(SS20)[:, None] - lse_t)
    CE = np.log(SS1) - g_s

    w2 = (gathered + shift) / max_preds
    losses = (1.0 - w2) * CE[:, None] + w2 * KD
    return np.asarray((thr * losses).sum(axis=1).mean(), dtype=np.float32)


def kernel(outputs1, outputs2, outputs3, outputs4, out_s, targets,
           _trace=False, _return_results=False):
    import ml_dtypes

    xs32 = [np.ascontiguousarray(np.asarray(a, dtype=np.float32))
            for a in (outputs1, outputs2, outputs3, outputs4)]
    s32 = np.ascontiguousarray(np.asarray(out_s, dtype=np.float32))
    tg = np.asarray(targets).astype(np.int64)

    idx = np.arange(B_FULL)
    g = np.stack([x[idx, tg] for x in xs32], axis=1).astype(np.float64)  # [B,4]
    g_s = s32[idx, tg].astype(np.float64)

    bf = ml_dtypes.bfloat16
    xs = [x.astype(bf) for x in xs32]
    s = s32.astype(bf)

    ident = np.eye(P, dtype=np.float32).astype(bf)
    negid = (-np.eye(P, dtype=np.float32)).astype(bf)
    in_maps = []
    for c in range(N_CORES):
        sl = slice(c * B_LOC, (c + 1) * B_LOC)
        m = {f"x{t}": xs[t][sl] for t in range(4)}
        m["s"] = s[sl]
        m["ident"] = ident
        m["negid"] = negid
        in_maps.append(m)

    results = _run_device(in_maps, trace=_trace)
    res_cores = [results.results[c]["res"] for c in range(N_CORES)]
    out = _host_combine(res_cores, g, g_s)
    if _return_results:
        return out, results
    return out


# revision 12
# speedup vs baseline: 1.1561x; 1.1561x over previous
"""Trainium2 Bass kernel for the Dynamic MultiTeacher distillation loss.

Strategy v4 (data-parallel over 8 NeuronCores, 1024 rows each), device
data in bf16 (host converts; tolerance is 2e-2, device noise ~1e-4):

Per 128-row tile:
  - DMA: s, x1..x4 bf16 (halves HBM traffic).
  - PE: mimS = x1+x2+x3+x4, psd1 = x1-s, psd2 = x2-s (PSUM f32 via
    identity matmuls; exact on bf16 inputs).
  - ACT: the 7 exp passes (only engine with exp); all 7 row-sums ride
    as fused accums (SS1, SS20, S1..S4, Sm).
  - DVE: top-2: x1/x2 via pairwise-max folds + max8(250), x3/x4/mimS
    via direct max8; dots D1..D4, DmA as single-pass stt-accum; DmB as
    tensor_scalar accum over the Pool-built em*s product.
  - Pool (GPSIMD, SBUF add/sub/mult only): diff3, diff4, em*s product.

All SBUF tiles are padded to 64B-multiple pitches so every ring buffer
lands on the same alignment class (DVE 2x/4x perf modes are
alignment-sensitive).

Host (tiny O(B) work in f64): exact gathers from the f32 originals,
global min/max scalars, margins = relu(gathered - m2), threshold
softmax, KD_t = T*D_t/S_t + T^2*(lse_s - lse_t), CE, final mean.
"""

import numpy as np

N_CORES = 8
B_FULL = 8192
C_DIM = 1000
CP = 1024                          # padded tile pitch (elements)
B_LOC = B_FULL // N_CORES          # 1024 rows per core
P = 128                            # partitions
N_TILES = B_LOC // P               # 8 row-tiles per core

T_KD = 20.0
T_THR = 6.0
EPS = 1e-05

# device output column layout: [P, 55] f32 (tile padded to 64 cols)
#   cols 8t..8t+7 : top8 of stream t (t=0..3: x_{t+1}; t=4: mimS=4*mimic)
#   40 SS1  = sum exp(s)
#   41 SS20 = sum exp(s/20)
#   42..45  : S_t = sum exp(x_t/20), t=1..4
#   46 Sm   = sum exp(mimS/80)
#   47..50  : D_t = sum e_t*(x_t - s), t=1..4
#   51 DmA  = sum em*mimS      (host: D_m = DmA/4 - DmB)
#   52 DmB  = sum em*s
OUT_COLS = 55

_CACHE = {}


def _build_nc():
    import concourse.bacc as bacc
    import concourse.mybir as mybir
    from concourse import tile

    nc = bacc.Bacc(
        "TRN2",
        target_bir_lowering=False,
        debug=False,
        num_devices=N_CORES,
    )
    f32 = mybir.dt.float32
    bf16 = mybir.dt.bfloat16
    Alu = mybir.AluOpType
    Act = mybir.ActivationFunctionType

    xs = [
        nc.dram_tensor(f"x{t}", [B_LOC, C_DIM], bf16, kind="ExternalInput").ap()
        for t in range(4)
    ]
    s_dram = nc.dram_tensor("s", [B_LOC, C_DIM], bf16, kind="ExternalInput").ap()
    ident = nc.dram_tensor("ident", [P, P], bf16, kind="ExternalInput").ap()
    negid = nc.dram_tensor("negid", [P, P], bf16, kind="ExternalInput").ap()
    res = nc.dram_tensor("res", [B_LOC, OUT_COLS], f32, kind="ExternalOutput").ap()

    with tile.TileContext(nc) as tc:
        with (
            tc.tile_pool(name="const", bufs=1) as cpool,
            tc.tile_pool(name="io", bufs=3) as xpool,
            tc.tile_pool(name="exps", bufs=2) as epool,
            tc.tile_pool(name="work", bufs=2) as wpool,
            tc.tile_pool(name="sink", bufs=2) as spool,
            tc.tile_pool(name="outs", bufs=3) as opool,
            tc.tile_pool(name="psm", bufs=2, space="PSUM") as pmpool,
            tc.tile_pool(name="psd", bufs=1, space="PSUM") as psdpool,
        ):
            id_tile = cpool.tile([P, P], bf16, tag="id")
            nc.sync.dma_start(out=id_tile[:], in_=ident)
            nid_tile = cpool.tile([P, P], bf16, tag="nid")
            nc.sync.dma_start(out=nid_tile[:], in_=negid)

            for i in range(N_TILES):
                r0 = i * P
                st_ = xpool.tile([P, CP], bf16, tag="s")
                st = st_[:, 0:C_DIM]
                nc.sync.dma_start(out=st, in_=s_dram[r0 : r0 + P, :])
                xt = []
                for t in range(4):
                    x_ = xpool.tile([P, CP], bf16, tag=f"x{t}")
                    x = x_[:, 0:C_DIM]
                    nc.sync.dma_start(out=x, in_=xs[t][r0 : r0 + P, :])
                    xt.append(x)

                out_t = opool.tile([P, 64], f32)

                # ---- ACT: student lse sums (only need s) ----
                sink1 = spool.tile([P, CP], bf16, tag="sink1")
                nc.scalar.activation(
                    sink1[:, 0:C_DIM], st, Act.Exp, scale=1.0,
                    accum_out=out_t[:, 40:41],
                )
                sink2 = spool.tile([P, CP], bf16, tag="sink2")
                nc.scalar.activation(
                    sink2[:, 0:C_DIM], st, Act.Exp, scale=1.0 / T_KD,
                    accum_out=out_t[:, 41:42],
                )

                # ---- PE: psd1 = x1-s, psd2 = x2-s, mimS = sum(x) ----
                psd1 = psdpool.tile([P, C_DIM], f32, tag="psd1")
                mims = pmpool.tile([P, C_DIM], f32, tag="mims")
                for c0, c1 in ((0, 512), (512, C_DIM)):
                    for t in range(4):
                        nc.tensor.matmul(
                            mims[:, c0:c1], id_tile[:], xt[t][:, c0:c1],
                            start=(t == 0), stop=(t == 3),
                        )
                for c0, c1 in ((0, 512), (512, C_DIM)):
                    nc.tensor.matmul(
                        psd1[:, c0:c1], id_tile[:], xt[0][:, c0:c1],
                        start=True, stop=False,
                    )
                    nc.tensor.matmul(
                        psd1[:, c0:c1], nid_tile[:], st[:, c0:c1],
                        start=False, stop=True,
                    )

                # ---- ACT: teacher exps (all carry accums) ----
                e_tiles = []
                for t in range(4):
                    e_ = epool.tile([P, CP], bf16, tag=f"e{t}")
                    e = e_[:, 0:C_DIM]
                    nc.scalar.activation(
                        e, xt[t], Act.Exp, scale=1.0 / T_KD,
                        accum_out=out_t[:, 42 + t : 43 + t],
                    )
                    e_tiles.append(e)
                em_ = epool.tile([P, CP], bf16, tag="em")
                em = em_[:, 0:C_DIM]
                nc.scalar.activation(
                    em, mims[:], Act.Exp, scale=1.0 / (4.0 * T_KD),
                    accum_out=out_t[:, 46:47],
                )

                # ---- DVE: top-2 via direct max8 ----
                for t in range(4):
                    nc.vector.max(out=out_t[:, 8 * t : 8 * t + 8], in_=xt[t])
                nc.vector.max(out=out_t[:, 32:40], in_=mims[:])

                # ---- Pool: diffs for teachers 2,3,4 (SBUF tt only) ----
                diffs = {}
                for t in (1, 2, 3):
                    d_ = wpool.tile([P, CP], bf16, tag=f"df{t}")
                    d = d_[:, 0:C_DIM]
                    nc.gpsimd.tensor_tensor(
                        out=d, in0=xt[t], in1=st, op=Alu.subtract
                    )
                    diffs[t] = d

                # ---- DVE: dots ----
                dA = wpool.tile([P, 1], f32, tag="dot0")
                nc.vector.scalar_tensor_tensor(
                    out=dA.broadcast_to([P, C_DIM]), in0=e_tiles[0], scalar=0.0,
                    in1=psd1[:], op0=Alu.bypass, op1=Alu.mult,
                    accum_out=out_t[:, 47:48],
                )
                for t in (1, 2):
                    dB = wpool.tile([P, 1], f32, tag=f"dot{t}")
                    nc.vector.scalar_tensor_tensor(
                        out=dB.broadcast_to([P, C_DIM]), in0=e_tiles[t],
                        scalar=0.0, in1=diffs[t], op0=Alu.bypass, op1=Alu.mult,
                        accum_out=out_t[:, 47 + t : 48 + t],
                    )
                dC = wpool.tile([P, 1], f32, tag="dot3")
                nc.vector.scalar_tensor_tensor(
                    out=dC.broadcast_to([P, C_DIM]), in0=e_tiles[3], scalar=0.0,
                    in1=diffs[3], op0=Alu.bypass, op1=Alu.mult,
                    accum_out=out_t[:, 50:51],
                )
                dD = wpool.tile([P, 1], f32, tag="dotA")
                nc.vector.scalar_tensor_tensor(
                    out=dD.broadcast_to([P, C_DIM]), in0=em, scalar=0.0,
                    in1=mims[:], op0=Alu.bypass, op1=Alu.mult,
                    accum_out=out_t[:, 51:52],
                )
                dE = wpool.tile([P, 1], f32, tag="dotB")
                nc.vector.scalar_tensor_tensor(
                    out=dE.broadcast_to([P, C_DIM]), in0=em, scalar=0.0,
                    in1=st, op0=Alu.bypass, op1=Alu.mult,
                    accum_out=out_t[:, 52:53],
                )

                nc.sync.dma_start(out=res[r0 : r0 + P, :],
                                  in_=out_t[:, 0:OUT_COLS])

    nc.finalize()
    return nc


def _get_nc():
    if "nc" not in _CACHE:
        _CACHE["nc"] = _build_nc()
    return _CACHE["nc"]


def _run_device(in_maps, trace=False):
    from concourse.bass_utils import run_bass_kernel_spmd

    nc = _get_nc()
    return run_bass_kernel_spmd(
        nc, in_maps, core_ids=list(range(N_CORES)), trace=trace
    )


def _host_combine(res_cores, g, g_s):
    """res_cores: [N_CORES][B_LOC, OUT_COLS] f32; g: [B,4] gathered teacher
    logits (f64); g_s: [B] gathered student logits (f64)."""
    r = np.concatenate(res_cores, axis=0).astype(np.float64)  # [B, 55]

    g_m = g.mean(axis=1)                                     # mimic gathered
    gathered = np.concatenate([g, g_m[:, None]], axis=1)     # [B,5]

    m1 = r[:, [0, 8, 16, 24, 32]].copy()
    m2 = r[:, [1, 9, 17, 25, 33]].copy()
    m1[:, 4] *= 0.25
    m2[:, 4] *= 0.25

    SS1 = r[:, 40]
    SS20 = r[:, 41]
    S = r[:, 42:47]                                          # [B,5]
    D = np.empty((r.shape[0], 5))
    D[:, :4] = r[:, 47:51]
    D[:, 4] = r[:, 51] * 0.25 - r[:, 52]

    Cmin = g.min()
    shift = (-Cmin + EPS) if Cmin < 0 else 0.0

    margins = np.maximum(gathered - m2, 0.0)
    z = margins / T_THR
    z = z - z.max(axis=1, keepdims=True)
    ez = np.exp(z)
    thr = ez / ez.sum(axis=1, keepdims=True)

    max_preds = m1[:, :4].max() + shift

    lse_t = np.log(S)
    KD = T_KD * D / S + (T_KD * T_KD) * (np.log(SS20)[:, None] - lse_t)
    CE = np.log(SS1) - g_s

    w2 = (gathered + shift) / max_preds
    losses = (1.0 - w2) * CE[:, None] + w2 * KD
    return np.asarray((thr * losses).sum(axis=1).mean(), dtype=np.float32)


def kernel(outputs1, outputs2, outputs3, outputs4, out_s, targets,
           _trace=False, _return_results=False):
    import ml_dtypes

    xs32 = [np.ascontiguousarray(np.asarray(a, dtype=np.float32))
            for a in (outputs1, outputs2, outputs3, outputs4)]
    s32 = np.ascontiguousarray(np.asarray(out_s, dtype=np.float32))
    tg = np.asarray(targets).astype(np.int64)

    idx = np.arange(B_FULL)
    g = np.stack([x[idx, tg] for x in xs32], axis=1).astype(np.float64)  # [B,4]
    g_s = s32[idx, tg].astype(np.float64)

    bf = ml_dtypes.bfloat16
    xs = [x.astype(bf) for x in xs32]
    s = s32.astype(bf)

    ident = np.eye(P, dtype=np.float32).astype(bf)
    negid = (-np.eye(P, dtype=np.float32)).astype(bf)
    in_maps = []
    for c in range(N_CORES):
        sl = slice(c * B_LOC, (c + 1) * B_LOC)
        m = {f"x{t}": xs[t][sl] for t in range(4)}
        m["s"] = s[sl]
        m["ident"] = ident
        m["negid"] = negid
        in_maps.append(m)

    results = _run_device(in_maps, trace=_trace)
    res_cores = [results.results[c]["res"] for c in range(N_CORES)]
    out = _host_combine(res_cores, g, g_s)
    if _return_results:
        return out, results
    return out
